# revision 30
# baseline (speedup 1.0000x reference)
"""Trainium2 Bass kernel for the Social Aggregator GNN message-passing module.

out[b, :] = sum_k att[b, k] * emb[u_u[nodes[b], k], :]
att[b, k] = 1 / (sqrt(deg[nodes[b]]) * sqrt(deg[u_u[nodes[b], k]])), 0 where inf

Sharding: data-parallel over the nodes batch dim across 8 NeuronCores
(2048 nodes per core); u_u and the embedding/degree tables are replicated.

Host-side staging is layout-only: indices cast to int32; emb/u_u_l are
concatenated into one [N_USERS, 65] table so a single 260B indirect-DMA
descriptor fetches both a neighbor's embedding row and its degree; u_u and
the u_u_l bit pattern are concatenated into a [N_USERS, 51] table so one
204B descriptor fetches a node's neighbor list and its own degree.

Raw-bass implementation (explicit per-engine pipelines with standalone
wait_ge instructions): walrus can encode at most one attached sync wait on
Pool DMA / 3D-AP instructions, so Tile's attached-wait style does not
compile for this instruction mix.

Pipeline (per 128-node tile t, 16 tiles):
  pool:  merged neighbor-list+degree gathers upfront, then eaug row
         gathers (4-deep buffering, spread over 4 SWDGE queues)
  dve:   copy deg column -> act sqrts -> attention weights -> weighted
         rows -> reduce over k
  act:   sqrt(deg) for centers and neighbors
  sync:  nodes load, per-tile output store
"""

from contextlib import ExitStack

import numpy as np

import concourse.bass as bass
import concourse.mybir as mybir
from concourse.bass_utils import run_bass_kernel_spmd

N_CORES = 8
BATCH = 16384
K = 50
D = 64
AUG = D + 1  # embedding row + degree column
N_USERS = 100000
P = 128  # SBUF partitions = nodes per tile
EB = 4  # eaug double-buffer depth

FP32 = mybir.dt.float32
I32 = mybir.dt.int32

N_SWDGE_QUEUES = 4  # spread descriptor generation over all queue contexts


def build_nc(b_loc: int = BATCH // N_CORES) -> bass.Bass:
    n_tiles = b_loc // P
    assert b_loc % P == 0
    eb = min(EB, n_tiles)

    nc = bass.Bass(
        "TRN2",
        target_bir_lowering=False,
        debug=False,
        num_swdge_queues=N_SWDGE_QUEUES,
    )
    nodes_d = nc.dram_tensor("nodes", [b_loc], I32, kind="ExternalInput")
    uu_d = nc.dram_tensor("uu", [N_USERS, K + 1], I32, kind="ExternalInput")
    aug_d = nc.dram_tensor("aug", [N_USERS, AUG], FP32, kind="ExternalInput")
    out_d = nc.dram_tensor("out", [b_loc, D], FP32, kind="ExternalOutput")

    # device-node j = p * n_tiles + t  (partition p, tile t)
    nodes_pt = nodes_d.ap().rearrange("(p t) -> p t", p=P)
    out_ptd = out_d.ap().rearrange("(p t) d -> p t d", p=P)

    with ExitStack() as ctx:
        nodes_sb = ctx.enter_context(nc.sbuf_tensor([P, n_tiles], I32))
        adj_all = ctx.enter_context(nc.sbuf_tensor([P, n_tiles, K + 1], I32))
        eaug_all = ctx.enter_context(nc.sbuf_tensor([P, eb, K, AUG], FP32))
        dgb = ctx.enter_context(nc.sbuf_tensor([P, 2, K], FP32))
        sqb = ctx.enter_context(nc.sbuf_tensor([P, 2, K], FP32))
        sqa_all = ctx.enter_context(nc.sbuf_tensor([P, n_tiles], FP32))
        prod = ctx.enter_context(nc.sbuf_tensor([P, K], FP32))
        msk = ctx.enter_context(nc.sbuf_tensor([P, K], FP32))
        pmx = ctx.enter_context(nc.sbuf_tensor([P, K], FP32))
        rec = ctx.enter_context(nc.sbuf_tensor([P, K], FP32))
        att = ctx.enter_context(nc.sbuf_tensor([P, K], FP32))
        wrow = ctx.enter_context(nc.sbuf_tensor([P, K, D], FP32))
        o_all = ctx.enter_context(nc.sbuf_tensor([P, n_tiles, D], FP32))

        s_nodes = ctx.enter_context(nc.semaphore("s_nodes"))
        s_idx = ctx.enter_context(nc.semaphore("s_idx"))
        # one semaphore per eaug slot: a shared counter would be racy under
        # out-of-order DMA completion
        s_eaug = [
            ctx.enter_context(nc.semaphore(f"s_eaug{i}")) for i in range(eb)
        ]
        s_act = ctx.enter_context(nc.semaphore("s_act"))
        s_out = ctx.enter_context(nc.semaphore("s_out"))
        s_dve = ctx.enter_context(nc.semaphore("s_dve"))

        # s_dve ticks are deterministic: per tile the DVE emits
        # [cp(t+1)] prod msk pmx att wrow reduce. Other engines wait on
        # specific tick values instead of dedicated semaphores (each
        # instruction may carry only one semaphore update).
        cp_tick: dict[int, int] = {}
        wrow_tick: dict[int, int] = {}
        red_tick: dict[int, int] = {}
        n = 0
        for t in range(n_tiles):
            if t == 0:
                n += 1
                cp_tick[0] = n
            if t + 1 < n_tiles:
                n += 1
                cp_tick[t + 1] = n
            n += 5  # prod, msk, pmx, rec, att
            n += 1
            wrow_tick[t] = n
            n += 1
            red_tick[t] = n

        block = ctx.enter_context(nc.Block())

        @block.sync
        def _(sync):
            sync.dma_start(nodes_sb[:], nodes_pt).then_inc(s_nodes, 16)
            for t in range(n_tiles):
                sync.wait_ge(s_dve, red_tick[t])
                out_ap = out_ptd[:, t : t + 1, :].rearrange("p o d -> p (o d)")
                sync.dma_start(out_ap, o_all[:, t, :]).then_inc(s_out, 16)
            sync.wait_ge(s_out, 16 * n_tiles)

        @block.gpsimd
        def _(gpsimd):
            gpsimd.wait_ge(s_nodes, 16)
            for t in range(n_tiles):
                idx_a = nodes_sb[:, t : t + 1]
                inst = gpsimd.indirect_dma_start(
                    out=adj_all[:, t, :],
                    out_offset=None,
                    in_=uu_d.ap(),
                    in_offset=bass.IndirectOffsetOnAxis(ap=idx_a, axis=0),
                )
                qn = t % N_SWDGE_QUEUES
                if qn:
                    inst.ins.queue = f"qPoolDynamic{qn}"
                inst.then_inc(s_idx, 16)
            gpsimd.wait_ge(s_idx, 16 * n_tiles)
            for t in range(n_tiles):
                if t >= eb:
                    gpsimd.wait_ge(s_dve, wrow_tick[t - eb])
                    gpsimd.wait_ge(s_eaug[t % eb], 16 * K * (t // eb))
                # the HW DGE expands one index per SBUF partition, so each
                # instruction gathers 128 rows: one neighbor slot k for the
                # tile's 128 center nodes
                for k in range(K):
                    inst = gpsimd.indirect_dma_start(
                        out=eaug_all[:, t % eb, k, :],
                        out_offset=None,
                        in_=aug_d.ap(),
                        in_offset=bass.IndirectOffsetOnAxis(
                            ap=adj_all[:, t, k : k + 1], axis=0
                        ),
                    )
                    qn = k % N_SWDGE_QUEUES
                    if qn:
                        inst.ins.queue = f"qPoolDynamic{qn}"
                    inst.then_inc(s_eaug[t % eb], 16)

        @block.scalar
        def _(scalar):
            # all dga gathers complete before the first sqa
            scalar.wait_ge(s_idx, 16 * n_tiles)
            for t in range(n_tiles):
                scalar.wait_ge(s_dve, cp_tick[t])
                scalar.activation(
                    sqb[:, t % 2, :],
                    dgb[:, t % 2, :],
                    mybir.ActivationFunctionType.Sqrt,
                )
                # in-order retire: this inc also implies the sqb above is done
                scalar.activation(
                    sqa_all[:, t : t + 1],
                    adj_all[:, t, K : K + 1].bitcast(FP32),
                    mybir.ActivationFunctionType.Sqrt,
                ).then_inc(s_act, 1)

        @block.vector
        def _(vector):
            # The DVE pipeline has no same-engine hazard interlocks, so each
            # op waits for the previous DVE op to retire via s_dve. The deg
            # copy for tile t+1 is issued one tile ahead so ACT's sqrts
            # overlap tile t's heavy multiply/reduce.
            tick = 0

            def step(make_inst):
                nonlocal tick
                if tick > 0:
                    vector.wait_ge(s_dve, tick)
                inst = make_inst()
                inst.then_inc(s_dve, 1)
                tick += 1
                return inst

            def copy_deg(t):
                eaug = eaug_all[:, t % eb, :, :]
                degb = eaug[:, :, D : D + 1].rearrange("p k o -> p (k o)")
                vector.wait_ge(s_eaug[t % eb], 16 * K * (t // eb + 1))
                step(lambda: vector.tensor_copy(dgb[:, t % 2, :], degb))
                assert tick == cp_tick[t], (tick, cp_tick[t])

            copy_deg(0)
            for t in range(n_tiles):
                vector.wait_ge(s_act, t + 1)
                if t + 1 < n_tiles:
                    copy_deg(t + 1)
                eaug = eaug_all[:, t % eb, :, :]
                step(
                    lambda: vector.tensor_tensor(
                        out=prod[:],
                        in0=sqb[:, t % 2, :],
                        in1=sqa_all[:, t : t + 1].to_broadcast([P, K]),
                        op=mybir.AluOpType.mult,
                    )
                )
                # prod is 0 where either degree is 0, else >= 1:
                # att = min(prod,1) * recip(max(prod,1)) masks zero degrees
                step(lambda: vector.tensor_scalar_min(msk[:], prod[:], 1.0))
                step(lambda: vector.tensor_scalar_max(pmx[:], prod[:], 1.0))
                step(lambda: vector.reciprocal(rec[:], pmx[:]))
                step(
                    lambda: vector.tensor_tensor(
                        out=att[:],
                        in0=msk[:],
                        in1=rec[:],
                        op=mybir.AluOpType.mult,
                    )
                )
                step(
                    lambda: vector.tensor_tensor(
                        out=wrow[:],
                        in0=eaug[:, :, 0:D],
                        in1=att[:].to_broadcast([P, K, D]),
                        op=mybir.AluOpType.mult,
                    )
                )
                assert tick == wrow_tick[t], (tick, wrow_tick[t])
                step(
                    lambda: vector.reduce_sum(
                        out=o_all[:, t, :],
                        in_=wrow[:].rearrange("p k d -> p d k"),
                        axis=mybir.AxisListType.X,
                    )
                )
                assert tick == red_tick[t], (tick, red_tick[t])

    return nc


_NC_CACHE: dict[int, bass.Bass] = {}


def _get_nc(b_loc: int) -> bass.Bass:
    if b_loc not in _NC_CACHE:
        _NC_CACHE[b_loc] = build_nc(b_loc)
    return _NC_CACHE[b_loc]


def make_in_maps(nodes, u_u, u_u_l, emb):
    """Host-side staging: shard nodes, cast indices, build the aug table."""
    nodes32 = np.ascontiguousarray(np.asarray(nodes).astype(np.int32))
    uu32 = np.ascontiguousarray(
        np.concatenate(
            [
                np.asarray(u_u).astype(np.int32),
                np.asarray(u_u_l, np.float32).view(np.int32),
            ],
            axis=1,
        )
    )
    aug = np.ascontiguousarray(
        np.concatenate(
            [np.asarray(emb, np.float32), np.asarray(u_u_l, np.float32)], axis=1
        )
    )
    b_loc = nodes32.shape[0] // N_CORES
    return [
        {"nodes": nodes32[i * b_loc : (i + 1) * b_loc], "uu": uu32, "aug": aug}
        for i in range(N_CORES)
    ], b_loc


def kernel(nodes, u_u, u_u_l, emb):
    in_maps, b_loc = make_in_maps(nodes, u_u, u_u_l, emb)
    nc = _get_nc(b_loc)
    res = run_bass_kernel_spmd(nc, in_maps, core_ids=list(range(N_CORES)))
    return np.concatenate([res.results[i]["out"] for i in range(N_CORES)], axis=0)
